# revision 17
# baseline (speedup 1.0000x reference)
"""Trainium2 Bass kernel for nn_Encoder (B=4, S=2048, D=512, H=8 self-attention).

Sharding over 8 NeuronCores: core c -> (batch b = c//2, head-group hg = c%2).
Each core computes, for its batch and its 4 heads, the full attention block
plus a partial output projection y_part = attn_out @ Wo[group rows]. The host
sums the two partial y tensors per batch.

Key compaction: the key-padding mask kills ~half the keys (their attention
weight is exactly exp(-1e9) = 0). The host permutes each batch's sequence so
valid keys come first; K/V projections, scores, exp and attnV run only over
NKT = ceil(n_valid/128) key tiles instead of 16. Queries stay full (the
output rows are un-permuted on the host).

Device-side layout (contraction dim always on SBUF partitions):
  xT_kv/xT_q*       : host-permuted input, d on partitions; separate tiles per
                      consumer so a later DMA never carries a WAR hazard
                      against projection reads
  KT/QT per pair    : [128, SV] / [128, 1024/th] = [2 heads' e, s/t]
  scores sc[tw]     : [s, {h0 512t | h1 512t}] -> the two heads' matmuls for a
                      given t-window write ONE psum tile, so the scheduler
                      keeps the (row_grp=0, row_grp=64) pair adjacent
  V' [s, e + ones]  : appended ones column makes the softmax denominator fall
                      out of the attnV matmul (psum row 64) for free
  outT [he, t]      : exactly the lhsT layout the Wo projection wants

ACT (exp) is the bottleneck engine; phases are paced so it never starves:
projection/Wo psums share the av tag and are scheduled into per-phase slack,
attnV drains through an `at` backlog, normalize reads av psum directly and
uses reciprocal_approx_fast, and the tail broadcasts recip via a PE matmul
(PE is idle there) instead of 4 serial gpsimd broadcasts.
"""

import ml_dtypes
import numpy as np

import concourse.mybir as mybir
import concourse.tile as tile
from concourse import bacc
from concourse.bass_utils import run_bass_kernel_spmd

B, S, D, H = 4, 2048, 512, 8
DH = D // H          # 64
HPC = H // 2         # 4 heads per core
HE = HPC * DH        # 256 output-proj rows per core
T = S                # full query length per core
NDC = D // 128       # 4 contraction chunks for projections
MASK_NUM = 1.0e9
N_CORES = 8

f32 = mybir.dt.float32
bf16 = mybir.dt.bfloat16
EXP = mybir.ActivationFunctionType.Exp
SCALE = float(1.0 / np.sqrt(DH))


def build_nc(nkt):
    SV = nkt * 128
    nc = bacc.Bacc("TRN2", target_bir_lowering=False, debug=False, num_devices=1)

    xT = nc.dram_tensor("xT", [D, S], bf16, kind="ExternalInput").ap()
    wq = nc.dram_tensor("wq", [D, HE], bf16, kind="ExternalInput").ap()
    wk = nc.dram_tensor("wk", [D, HE], bf16, kind="ExternalInput").ap()
    wv = nc.dram_tensor("wv", [D, HE], bf16, kind="ExternalInput").ap()
    wo = nc.dram_tensor("wo", [HE, D], bf16, kind="ExternalInput").ap()
    mb = nc.dram_tensor("mbias", [SV], f32, kind="ExternalInput").ap()
    y = nc.dram_tensor("y", [T, D], f32, kind="ExternalOutput").ap()

    with tile.TileContext(nc) as tc:
        with (
            tc.tile_pool(name="const", bufs=1) as const,
            tc.tile_pool(name="psS", bufs=2, space="PSUM") as psS,
            tc.tile_pool(name="psA", bufs=4, space="PSUM") as psA,
            tc.tile_pool(name="attnT", bufs=18) as at_pool,
            tc.tile_pool(name="yout", bufs=4) as y_pool,
            tc.tile_pool(name="recip", bufs=4) as r_pool,
            tc.tile_pool(name="recipb", bufs=4) as rb_pool,
            tc.tile_pool(name="sums", bufs=2) as sums_pool,
            tc.tile_pool(name="avsb", bufs=8) as avsb_pool,
        ):
            # ---- HAM warmup: keep PE busy during the DMA prologue so the
            # clock gate is at 8/8 when real matmuls arrive.
            warm_sb = const.tile([128, 512], bf16, tag="warm")
            nc.gpsimd.memset(warm_sb[:], 0.0)
            warm_ps = psA.tile([128, 512], f32, tag="mm", name="warm_ps")
            for _ in range(18):
                nc.tensor.matmul(
                    warm_ps[:], lhsT=warm_sb[:, 0:128], rhs=warm_sb[:],
                    start=True, stop=True,
                )

            # ---- ACT table preload: a dummy exp so the Exp spline tables DMA
            # in during the prologue instead of before the first real exp.
            warm_act = const.tile([1, 2], f32, tag="warm_act")
            nc.gpsimd.memset(warm_act[:], 0.0)
            nc.scalar.activation(warm_act[0:1, 0:1], warm_act[0:1, 1:2], EXP)

            # ---- DMA loads, critical-path order -------------------------
            # Column-sliced into separate tiles so the first projections can
            # start as soon as their slice lands (dep granularity is per-tile).
            kv_slices = []
            c0 = 0
            while c0 < SV:
                w = min(512, SV - c0)
                kv_slices.append((c0, w))
                c0 += w
            xT_kv = [
                const.tile([128, NDC, w], bf16, tag=f"xTkv{j}", name=f"xTkv{j}")
                for j, (c0, w) in enumerate(kv_slices)
            ]
            xT_q = [
                [
                    const.tile(
                        [128, NDC, 512], bf16, tag=f"xTq{th}{i}", name=f"xTq{th}{i}"
                    )
                    for i in range(2)
                ]
                for th in range(2)
            ]
            wq_sb = const.tile([128, NDC, HE], bf16, tag="wq")
            wk_sb = const.tile([128, NDC, HE], bf16, tag="wk")
            wv_sb = const.tile([128, NDC, HE], bf16, tag="wv")
            wo_sb = const.tile([128, HE // 128, D], bf16, tag="wo")
            mb_sb = const.tile([128, nkt], f32, tag="mb")
            xT_r = xT.rearrange("(c p) s -> c p s", p=128)

            def dma_kv(j):
                c0, w = kv_slices[j]
                for dc in range(NDC):
                    nc.sync.dma_start(xT_kv[j][:, dc, :], xT_r[dc][:, c0 : c0 + w])

            def dma_q(th, i):
                t0 = th * 1024 + i * 512
                for dc in range(NDC):
                    nc.sync.dma_start(
                        xT_q[th][i][:, dc, :], xT_r[dc][:, t0 : t0 + 512]
                    )

            nc.sync.dma_start(wk_sb[:], wk.rearrange("(c p) n -> p c n", p=128))
            dma_kv(0)
            nc.sync.dma_start(wq_sb[:], wq.rearrange("(c p) n -> p c n", p=128))
            dma_q(0, 0)
            nc.sync.dma_start(mb_sb[:], mb.rearrange("(j p) -> p j", p=128))
            dma_kv(1)
            dma_q(0, 1)
            nc.sync.dma_start(wv_sb[:], wv.rearrange("(c p) n -> p c n", p=128))
            for j in range(2, len(kv_slices)):
                dma_kv(j)
            dma_q(1, 0)
            dma_q(1, 1)
            nc.sync.dma_start(wo_sb[:], wo.rearrange("(c p) n -> p c n", p=128))

            # V' tiles: [s-tile][local head][DH + ones column]
            v_sb = const.tile([128, nkt, HPC, DH + 1], bf16, tag="v")
            nc.gpsimd.memset(v_sb[:, :, :, DH : DH + 1], 1.0)
            ones_sb = const.tile([1, DH], bf16, tag="ones")
            nc.gpsimd.memset(ones_sb[:], 1.0)

            kt_sb = [
                const.tile([128, SV], bf16, tag=f"kt{pp}", name=f"kt{pp}")
                for pp in range(2)
            ]
            qt_sb = [
                const.tile([128, T], bf16, tag=f"qt{pp}", name=f"qt{pp}")
                for pp in range(2)
            ]
            outT_sb = const.tile([128, HE // 128, T], bf16, tag="outT")

            # ---- projection emitters (psum shares the "mm" tag) ----------
            def emit_kt(pp, slices=None):
                for j in slices if slices is not None else range(len(kv_slices)):
                    c0, w = kv_slices[j]
                    ps = psA.tile([128, 512], f32, tag="mm", name="kproj_ps")
                    for dc in range(NDC):
                        nc.tensor.matmul(
                            ps[:, 0:w],
                            lhsT=wk_sb[:, dc, pp * 128 : (pp + 1) * 128],
                            rhs=xT_kv[j][:, dc, :],
                            start=(dc == 0),
                            stop=(dc == NDC - 1),
                        )
                    nc.vector.tensor_copy(kt_sb[pp][:, c0 : c0 + w], ps[:, 0:w])

            def emit_qt(pp, th, sc_is=(0, 1)):
                for sc_i in sc_is:
                    ps = psA.tile([128, 512], f32, tag="mm", name="qproj_ps")
                    for dc in range(NDC):
                        nc.tensor.matmul(
                            ps[:],
                            lhsT=wq_sb[:, dc, pp * 128 : (pp + 1) * 128],
                            rhs=xT_q[th][sc_i][:, dc, :],
                            start=(dc == 0),
                            stop=(dc == NDC - 1),
                        )
                    t0 = th * 1024 + sc_i * 512
                    nc.vector.tensor_copy(qt_sb[pp][:, t0 : t0 + 512], ps[:])

            def emit_v(vst):
                j, r = vst // 4, vst % 4
                ps = psA.tile([128, 512], f32, tag="mm", name="vproj_ps")
                for dc in range(NDC):
                    nc.tensor.matmul(
                        ps[:, 0:HE],
                        lhsT=xT_kv[j][:, dc, r * 128 : (r + 1) * 128],
                        rhs=wv_sb[:, dc, :],
                        start=(dc == 0),
                        stop=(dc == NDC - 1),
                    )
                nc.vector.tensor_copy(
                    v_sb[:, vst, :, 0:DH],
                    ps[:, 0:HE].rearrange("p (h e) -> p h e", e=DH),
                )

            # ---- attention emitters --------------------------------------
            # sc tile per (st, tw): cols 0:512 = h0, 512:1024 = h1. The two
            # heads' matmuls write one tile -> adjacent in the PE queue ->
            # concurrent in row groups 0/64.
            def emit_scores(pp, th, st):
                ats = []
                for tw in range(2):
                    scp = psS.tile([128, 1024], f32, tag="sc", name=f"sc{tw}")
                    tcol = th * 1024 + tw * 512
                    for h2 in range(2):
                        off = h2 * 64
                        nc.tensor.matmul(
                            scp[:, h2 * 512 : (h2 + 1) * 512],
                            lhsT=kt_sb[pp][off : off + 64, st * 128 : (st + 1) * 128],
                            rhs=qt_sb[pp][off : off + 64, tcol : tcol + 512],
                            start=True,
                            stop=True,
                        )
                    at = at_pool.tile([128, 1024], bf16, tag="at", name="at")
                    nc.scalar.activation(
                        at[:], scp[:], EXP,
                        bias=mb_sb[:, st : st + 1], scale=SCALE,
                    )
                    ats.append(at)
                return ats

            def emit_attnv(av_l, at, st, pp, tw):
                for h2 in range(2):
                    h = 2 * pp + h2
                    nc.tensor.matmul(
                        av_l[h2][tw][0 : DH + 1, :],
                        lhsT=v_sb[:, st, h, :],
                        rhs=at[:, h2 * 512 : (h2 + 1) * 512],
                        start=(st == 0),
                        stop=(st == nkt - 1),
                    )

            # stage the pending phase's av psum to SBUF: releases the psA
            # slots immediately so boundary fillers never head-block the PE
            # queue behind the (slow, gpsimd-serial) normalize chain
            def emit_stage(av_):
                staged = {}
                for h2 in range(2):
                    for tw in range(2):
                        s = avsb_pool.tile(
                            [DH + 1, 512], f32, tag="avsb", name="av_sb"
                        )
                        nc.vector.tensor_copy(s[:], av_[h2][tw][0 : DH + 1, :])
                        staged[(h2, tw)] = s
                return staged

            # Boundary normalize: everything after the recip runs on GPSIMD
            # (broadcast + multiply). Keeping the pb-gated multiplies OFF the
            # DVE FIFO is critical — any DVE op queued behind them (psum
            # drains of boundary projections) would stall the whole phase.
            def emit_normalize(th_, pp_, get):
                sums = sums_pool.tile([97, 512], f32, tag="sums", name="sums")
                nc.gpsimd.memset(sums[:], 1.0)
                for h2 in range(2):
                    for tw in range(2):
                        r = 32 * (2 * tw + h2)
                        nc.vector.tensor_copy(
                            sums[r : r + 1, :], get(h2, tw)[DH : DH + 1, :]
                        )
                recips = sums_pool.tile([97, 512], f32, tag="recips", name="recips")
                nc.vector.reciprocal_approx_fast(recips[:], sums[:])
                for tw in range(2):
                    for h2 in range(2):
                        r = 32 * (2 * tw + h2)
                        tcol = th_ * 1024 + tw * 512
                        r_t = r_pool.tile([1, 512], f32, tag="r", name="r_t")
                        nc.vector.tensor_copy(r_t[0:1, :], recips[r : r + 1, :])
                        rb_t = rb_pool.tile([64, 512], f32, tag="rb", name="rb_t")
                        nc.gpsimd.partition_broadcast(rb_t[:], r_t[0:1, :])
                        nc.gpsimd.tensor_mul(
                            outT_sb[h2 * 64 : (h2 + 1) * 64, pp_, tcol : tcol + 512],
                            get(h2, tw)[0:DH, :],
                            rb_t[:],
                        )

            def emit_wo(th_, tts=None):
                for tt in tts if tts is not None else range(th_ * 8, (th_ + 1) * 8):
                    ps = psA.tile([128, 512], f32, tag="mm", name="y_ps")
                    for c in range(HE // 128):
                        nc.tensor.matmul(
                            ps[:],
                            lhsT=outT_sb[:, c, tt * 128 : (tt + 1) * 128],
                            rhs=wo_sb[:, c, :],
                            start=(c == 0),
                            stop=(c == HE // 128 - 1),
                        )
                    y_sb = y_pool.tile([128, 512], f32, tag="y", name="y_sb")
                    nc.vector.tensor_copy(y_sb[:], ps[:])
                    nc.sync.dma_start(y[tt * 128 : (tt + 1) * 128, :], y_sb[:])

            # ---- prologue: projections chase the DMA slices --------------
            emit_kt(0, [0])
            emit_qt(0, 0, (0,))
            emit_kt(0, [1])
            emit_qt(0, 0, (1,))
            emit_kt(0, list(range(2, len(kv_slices))))

            # ---- phases: pp0 first so kt1/qt1 projections spread out -----
            phases = [(0, 0), (1, 0), (0, 1), (1, 1)]
            pending = None

            filler_plan = {}

            def add_filler(ph, st, fn):
                filler_plan.setdefault((ph, min(st, nkt - 1)), []).append(fn)

            v_jobs = [(lambda vst=vst: emit_v(vst)) for vst in range(nkt)]
            slots0 = max(min(5, nkt - 1), 1)
            per = (len(v_jobs) + slots0 - 1) // slots0
            for si in range(slots0):
                for fn in v_jobs[si * per : (si + 1) * per]:
                    add_filler(0, si, fn)
            add_filler(0, 5, lambda: emit_qt(0, 1))
            add_filler(1, 2, lambda: emit_kt(1))
            add_filler(1, 3, lambda: emit_qt(1, 0))
            add_filler(2, 2, lambda: emit_qt(1, 1))
            add_filler(3, 6, lambda: emit_wo(0, range(0, 4)))
            add_filler(3, 7, lambda: emit_wo(0, range(4, 8)))
            av_st = {
                0: min(5, nkt - 1),
                1: min(3, nkt - 1),
                2: min(2, nkt - 1),
                3: min(7, nkt - 1),
            }

            for phase_i, (th, pp) in enumerate(phases):
                av_l = None
                backlog = []

                def ensure_av():
                    nonlocal av_l
                    if av_l is not None:
                        return
                    av_l = [
                        [
                            psA.tile(
                                [128, 512], f32, tag="mm", name=f"av{h2}_{tw}"
                            )
                            for tw in range(2)
                        ]
                        for h2 in range(2)
                    ]
                    for at_, st_, tw_ in backlog:
                        emit_attnv(av_l, at_, st_, pp, tw_)
                    backlog.clear()

                staged = None
                for st in range(nkt):
                    ats = emit_scores(pp, th, st)
                    for tw, at in enumerate(ats):
                        if av_l is None:
                            backlog.append((at, st, tw))
                        else:
                            emit_attnv(av_l, at, st, pp, tw)
                    if st == 0 and pending is not None:
                        staged = emit_stage(pending[2])
                    if st == 1 and pending is not None:
                        emit_normalize(
                            pending[0], pending[1],
                            lambda h2, tw: staged[(h2, tw)],
                        )
                        pending = None
                    for fn in filler_plan.get((phase_i, st), []):
                        fn()
                    if st == av_st[phase_i]:
                        ensure_av()
                ensure_av()
                pending = (th, pp, av_l)

            # ---- tail: last phase normalize + Wo -------------------------
            # PE is idle here: broadcast recip rows via a ones[1,64] matmul
            # into free sc-tag psum instead of 4 serial gpsimd broadcasts.
            # Batched emission (copies, then broadcasts, then muls) so the
            # DVE/PE queues never ping-pong.
            th_, pp_, av_ = pending
            sums = sums_pool.tile([97, 512], f32, tag="sums", name="sums")
            nc.gpsimd.memset(sums[:], 1.0)
            for h2 in range(2):
                for tw in range(2):
                    r = 32 * (2 * tw + h2)
                    nc.vector.tensor_copy(
                        sums[r : r + 1, :], av_[h2][tw][DH : DH + 1, :]
                    )
            recips = sums_pool.tile([97, 512], f32, tag="recips", name="recips")
            nc.vector.reciprocal_approx_fast(recips[:], sums[:])
            r_bfs = {}
            for tw in range(2):
                for h2 in range(2):
                    r = 32 * (2 * tw + h2)
                    r_bf = r_pool.tile([1, 512], bf16, tag="rbf", name="r_bf")
                    nc.vector.tensor_copy(r_bf[0:1, :], recips[r : r + 1, :])
                    r_bfs[(h2, tw)] = r_bf
            tail_staged = emit_stage(av_)
            rb_pss = {}
            for tw in range(2):
                for h2 in range(2):
                    rb_ps = psS.tile([64, 512], f32, tag="sc", name="rb_ps")
                    nc.tensor.matmul(
                        rb_ps[:], lhsT=ones_sb[:], rhs=r_bfs[(h2, tw)][0:1, :],
                        start=True, stop=True,
                    )
                    rb_pss[(h2, tw)] = rb_ps
            for tw in range(2):
                for h2 in range(2):
                    tcol = th_ * 1024 + tw * 512
                    nc.vector.tensor_mul(
                        outT_sb[h2 * 64 : (h2 + 1) * 64, pp_, tcol : tcol + 512],
                        tail_staged[(h2, tw)][0:DH, :],
                        rb_pss[(h2, tw)][:],
                    )
                emit_wo(th_, tts=range(th_ * 8 + tw * 4, th_ * 8 + (tw + 1) * 4))

    nc.compile()
    return nc


_NC_CACHE = {}
_LAST_STATE = {}


def _get_nc(nkt=None):
    if nkt is None:
        nkt = _LAST_STATE.get("nkt", 9)
    if nkt not in _NC_CACHE:
        _NC_CACHE[nkt] = build_nc(nkt)
    return _NC_CACHE[nkt]


def make_in_maps(x, mask, Wq, Wk, Wv, Wo):
    bf = ml_dtypes.bfloat16
    mask = np.asarray(mask)
    perms = []
    counts = []
    for b in range(B):
        valid = np.flatnonzero(mask[b] > 0)
        invalid = np.flatnonzero(mask[b] <= 0)
        perms.append(np.concatenate([valid, invalid]).astype(np.int64))
        counts.append(len(valid))
    nkt = max(1, int(np.ceil(max(counts) / 128)))
    nkt = min(nkt, S // 128)
    SV = nkt * 128
    _LAST_STATE["nkt"] = nkt
    _LAST_STATE["perms"] = perms

    # [H, D, DH] -> [D, H*DH]
    wq_f = np.ascontiguousarray(Wq.transpose(1, 0, 2).reshape(D, H * DH))
    wk_f = np.ascontiguousarray(Wk.transpose(1, 0, 2).reshape(D, H * DH))
    wv_f = np.ascontiguousarray(Wv.transpose(1, 0, 2).reshape(D, H * DH))
    in_maps = []
    for c in range(N_CORES):
        b, hg = c // 2, c % 2
        perm = perms[b]
        xT_p = np.ascontiguousarray(x[b].T[:, perm]).astype(bf)  # [D, S] permuted
        mb = np.where(mask[b][perm] > 0, 0.0, -MASK_NUM).astype(np.float32)[:SV]
        cols = slice(hg * HE, (hg + 1) * HE)
        in_maps.append(
            {
                "xT": xT_p,
                "wq": np.ascontiguousarray(wq_f[:, cols]).astype(bf),
                "wk": np.ascontiguousarray(wk_f[:, cols]).astype(bf),
                "wv": np.ascontiguousarray(wv_f[:, cols]).astype(bf),
                "wo": np.ascontiguousarray(Wo[cols, :]).astype(bf),
                "mbias": np.ascontiguousarray(mb),
            }
        )
    return in_maps


def combine_results(results):
    perms = _LAST_STATE["perms"]
    y = np.zeros((B, S, D), np.float32)
    for b in range(B):
        yp = results[2 * b]["y"] + results[2 * b + 1]["y"]
        y[b][perms[b]] = yp
    return y


def kernel(x, mask, Wq, Wk, Wv, Wo):
    in_maps = make_in_maps(
        np.asarray(x, np.float32),
        np.asarray(mask),
        np.asarray(Wq, np.float32),
        np.asarray(Wk, np.float32),
        np.asarray(Wv, np.float32),
        np.asarray(Wo, np.float32),
    )
    nc = _get_nc(_LAST_STATE["nkt"])
    res = run_bass_kernel_spmd(nc, in_maps, core_ids=list(range(N_CORES)))
    return combine_results(res.results)


# revision 19
# speedup vs baseline: 1.3894x; 1.3894x over previous
"""Trainium2 Bass kernel for nn_Encoder (B=4, S=2048, D=512, H=8 self-attention).

Sharding over 8 NeuronCores: core c -> (batch b = c//2, head-group hg = c%2).
Each core computes, for its batch and its 4 heads, the full attention block
plus a partial output projection y_part = attn_out @ Wo[group rows]. The host
sums the two partial y tensors per batch.

Key compaction: the key-padding mask kills ~half the keys (their attention
weight is exactly exp(-1e9) = 0). The host permutes each batch's sequence so
valid keys come first; K/V projections, scores, exp and attnV run only over
NKT = ceil(n_valid/128) key tiles instead of 16. Queries stay full (the
output rows are un-permuted on the host).

Device-side layout (contraction dim always on SBUF partitions):
  xT_kv/xT_q*       : host-permuted input, d on partitions; separate tiles per
                      consumer so a later DMA never carries a WAR hazard
                      against projection reads
  KT/QT per pair    : [128, SV] / [128, 1024/th] = [2 heads' e, s/t]
  scores sc[tw]     : [s, {h0 512t | h1 512t}] -> the two heads' matmuls for a
                      given t-window write ONE psum tile, so the scheduler
                      keeps the (row_grp=0, row_grp=64) pair adjacent
  V' [s, e + ones]  : appended ones column makes the softmax denominator fall
                      out of the attnV matmul (psum row 64) for free
  outT [he, t]      : exactly the lhsT layout the Wo projection wants

ACT (exp) is the bottleneck engine; phases are paced so it never starves:
projection/Wo psums share the av tag and are scheduled into per-phase slack,
attnV drains through an `at` backlog, normalize reads av psum directly and
uses reciprocal_approx_fast, and the tail broadcasts recip via a PE matmul
(PE is idle there) instead of 4 serial gpsimd broadcasts.
"""

import ml_dtypes
import numpy as np

import concourse.mybir as mybir
import concourse.tile as tile
from concourse import bacc
from concourse.bass_utils import run_bass_kernel_spmd

B, S, D, H = 4, 2048, 512, 8
DH = D // H          # 64
HPC = H // 2         # 4 heads per core
HE = HPC * DH        # 256 output-proj rows per core
T = S                # full query length per core
NDC = D // 128       # 4 contraction chunks for projections
MASK_NUM = 1.0e9
N_CORES = 8

f32 = mybir.dt.float32
bf16 = mybir.dt.bfloat16
EXP = mybir.ActivationFunctionType.Exp
SCALE = float(1.0 / np.sqrt(DH))


def build_nc(nkt):
    SV = nkt * 128
    nc = bacc.Bacc("TRN2", target_bir_lowering=False, debug=False, num_devices=1)

    xT = nc.dram_tensor("xT", [D, S], bf16, kind="ExternalInput").ap()
    wq = nc.dram_tensor("wq", [D, HE], bf16, kind="ExternalInput").ap()
    wk = nc.dram_tensor("wk", [D, HE], bf16, kind="ExternalInput").ap()
    wv = nc.dram_tensor("wv", [D, HE], bf16, kind="ExternalInput").ap()
    wo = nc.dram_tensor("wo", [HE, D], bf16, kind="ExternalInput").ap()
    mb = nc.dram_tensor("mbias", [SV], f32, kind="ExternalInput").ap()
    y = nc.dram_tensor("y", [T, D], f32, kind="ExternalOutput").ap()

    with tile.TileContext(nc) as tc:
        with (
            tc.tile_pool(name="const", bufs=1) as const,
            tc.tile_pool(name="psS", bufs=2, space="PSUM") as psS,
            tc.tile_pool(name="psA", bufs=4, space="PSUM") as psA,
            tc.tile_pool(name="attnT", bufs=18) as at_pool,
            tc.tile_pool(name="yout", bufs=4) as y_pool,
            tc.tile_pool(name="recip", bufs=4) as r_pool,
            tc.tile_pool(name="recipb", bufs=4) as rb_pool,
            tc.tile_pool(name="sums", bufs=2) as sums_pool,
            tc.tile_pool(name="avsb", bufs=8) as avsb_pool,
        ):
            # ---- HAM warmup: keep PE busy during the DMA prologue so the
            # clock gate is at 8/8 when real matmuls arrive.
            warm_sb = const.tile([128, 512], bf16, tag="warm")
            nc.gpsimd.memset(warm_sb[:], 0.0)
            warm_ps = psA.tile([128, 512], f32, tag="mm", name="warm_ps")
            for _ in range(18):
                nc.tensor.matmul(
                    warm_ps[:], lhsT=warm_sb[:, 0:128], rhs=warm_sb[:],
                    start=True, stop=True,
                )

            # ---- ACT table preload: a dummy exp so the Exp spline tables DMA
            # in during the prologue instead of before the first real exp.
            warm_act = const.tile([1, 2], f32, tag="warm_act")
            nc.gpsimd.memset(warm_act[:], 0.0)
            nc.scalar.activation(warm_act[0:1, 0:1], warm_act[0:1, 1:2], EXP)

            # ---- DMA loads, critical-path order -------------------------
            # Column-sliced into separate tiles so the first projections can
            # start as soon as their slice lands (dep granularity is per-tile).
            kv_slices = []
            c0 = 0
            while c0 < SV:
                w = min(512, SV - c0)
                kv_slices.append((c0, w))
                c0 += w
            xT_kv = [
                const.tile([128, NDC, w], bf16, tag=f"xTkv{j}", name=f"xTkv{j}")
                for j, (c0, w) in enumerate(kv_slices)
            ]
            xT_q = [
                [
                    const.tile(
                        [128, NDC, 512], bf16, tag=f"xTq{th}{i}", name=f"xTq{th}{i}"
                    )
                    for i in range(2)
                ]
                for th in range(2)
            ]
            wq_sb = const.tile([128, NDC, HE], bf16, tag="wq")
            wk_sb = const.tile([128, NDC, HE], bf16, tag="wk")
            wv_sb = const.tile([128, NDC, HE], bf16, tag="wv")
            wo_sb = const.tile([128, HE // 128, D], bf16, tag="wo")
            mb_sb = const.tile([128, nkt], f32, tag="mb")
            xT_r = xT.rearrange("(c p) s -> c p s", p=128)

            def dma_kv(j):
                c0, w = kv_slices[j]
                for dc in range(NDC):
                    nc.sync.dma_start(xT_kv[j][:, dc, :], xT_r[dc][:, c0 : c0 + w])

            def dma_q(th, i):
                t0 = th * 1024 + i * 512
                for dc in range(NDC):
                    nc.sync.dma_start(
                        xT_q[th][i][:, dc, :], xT_r[dc][:, t0 : t0 + 512]
                    )

            nc.sync.dma_start(wk_sb[:], wk.rearrange("(c p) n -> p c n", p=128))
            dma_kv(0)
            nc.sync.dma_start(wq_sb[:], wq.rearrange("(c p) n -> p c n", p=128))
            dma_q(0, 0)
            nc.sync.dma_start(mb_sb[:], mb.rearrange("(j p) -> p j", p=128))
            dma_kv(1)
            dma_q(0, 1)
            nc.sync.dma_start(wv_sb[:], wv.rearrange("(c p) n -> p c n", p=128))
            for j in range(2, len(kv_slices)):
                dma_kv(j)
            dma_q(1, 0)
            dma_q(1, 1)
            nc.sync.dma_start(wo_sb[:], wo.rearrange("(c p) n -> p c n", p=128))

            # V' tiles: [s-tile][local head][DH + ones column]
            v_sb = const.tile([128, nkt, HPC, DH + 1], bf16, tag="v")
            nc.gpsimd.memset(v_sb[:, :, :, DH : DH + 1], 1.0)
            ones_sb = const.tile([1, DH], bf16, tag="ones")
            nc.gpsimd.memset(ones_sb[:], 1.0)

            kt_sb = [
                const.tile([128, SV], bf16, tag=f"kt{pp}", name=f"kt{pp}")
                for pp in range(2)
            ]
            qt_sb = [
                const.tile([128, T], bf16, tag=f"qt{pp}", name=f"qt{pp}")
                for pp in range(2)
            ]
            outT_sb = const.tile([128, HE // 128, T], bf16, tag="outT")

            # ---- projection emitters (psum shares the "mm" tag) ----------
            def emit_kt(pp, slices=None):
                for j in slices if slices is not None else range(len(kv_slices)):
                    c0, w = kv_slices[j]
                    ps = psA.tile([128, 512], f32, tag="mm", name="kproj_ps")
                    for dc in range(NDC):
                        nc.tensor.matmul(
                            ps[:, 0:w],
                            lhsT=wk_sb[:, dc, pp * 128 : (pp + 1) * 128],
                            rhs=xT_kv[j][:, dc, :],
                            start=(dc == 0),
                            stop=(dc == NDC - 1),
                        )
                    nc.vector.tensor_copy(kt_sb[pp][:, c0 : c0 + w], ps[:, 0:w])

            def emit_qt(pp, th, sc_is=(0, 1)):
                for sc_i in sc_is:
                    ps = psA.tile([128, 512], f32, tag="mm", name="qproj_ps")
                    for dc in range(NDC):
                        nc.tensor.matmul(
                            ps[:],
                            lhsT=wq_sb[:, dc, pp * 128 : (pp + 1) * 128],
                            rhs=xT_q[th][sc_i][:, dc, :],
                            start=(dc == 0),
                            stop=(dc == NDC - 1),
                        )
                    t0 = th * 1024 + sc_i * 512
                    nc.vector.tensor_copy(qt_sb[pp][:, t0 : t0 + 512], ps[:])

            def emit_v(vst):
                j, r = vst // 4, vst % 4
                ps = psA.tile([128, 512], f32, tag="mm", name="vproj_ps")
                for dc in range(NDC):
                    nc.tensor.matmul(
                        ps[:, 0:HE],
                        lhsT=xT_kv[j][:, dc, r * 128 : (r + 1) * 128],
                        rhs=wv_sb[:, dc, :],
                        start=(dc == 0),
                        stop=(dc == NDC - 1),
                    )
                nc.vector.tensor_copy(
                    v_sb[:, vst, :, 0:DH],
                    ps[:, 0:HE].rearrange("p (h e) -> p h e", e=DH),
                )

            # ---- attention emitters --------------------------------------
            # sc tile per (st, tw): cols 0:512 = h0, 512:1024 = h1. The two
            # heads' matmuls write one tile -> adjacent in the PE queue ->
            # concurrent in row groups 0/64.
            def emit_scores(pp, th, st):
                ats = []
                for tw in range(2):
                    scp = psS.tile([128, 1024], f32, tag="sc", name=f"sc{tw}")
                    tcol = th * 1024 + tw * 512
                    for h2 in range(2):
                        off = h2 * 64
                        nc.tensor.matmul(
                            scp[:, h2 * 512 : (h2 + 1) * 512],
                            lhsT=kt_sb[pp][off : off + 64, st * 128 : (st + 1) * 128],
                            rhs=qt_sb[pp][off : off + 64, tcol : tcol + 512],
                            start=True,
                            stop=True,
                        )
                    at = at_pool.tile([128, 1024], bf16, tag="at", name="at")
                    nc.scalar.activation(
                        at[:], scp[:], EXP,
                        bias=mb_sb[:, st : st + 1], scale=SCALE,
                    )
                    ats.append(at)
                return ats

            def emit_attnv(av_l, at, st, pp, tw):
                for h2 in range(2):
                    h = 2 * pp + h2
                    nc.tensor.matmul(
                        av_l[h2][tw][0 : DH + 1, :],
                        lhsT=v_sb[:, st, h, :],
                        rhs=at[:, h2 * 512 : (h2 + 1) * 512],
                        start=(st == 0),
                        stop=(st == nkt - 1),
                    )

            # stage the pending phase's av psum to SBUF: releases the psA
            # slots immediately so boundary fillers never head-block the PE
            # queue behind the (slow, gpsimd-serial) normalize chain
            def emit_stage(av_):
                staged = {}
                for h2 in range(2):
                    for tw in range(2):
                        s = avsb_pool.tile(
                            [DH + 1, 512], f32, tag="avsb", name="av_sb"
                        )
                        nc.vector.tensor_copy(s[:], av_[h2][tw][0 : DH + 1, :])
                        staged[(h2, tw)] = s
                return staged

            # Boundary normalize, split in two emission points: the recip +
            # gpsimd broadcasts go early (st1); the DVE multiplies are
            # emitted at st4 so the boundary projections' psum-drain copies
            # queue AHEAD of them in the DVE FIFO — by st4 the broadcasts are
            # long done and the muls never head-block anything.
            def emit_norm_recip(get):
                sums = sums_pool.tile([97, 512], f32, tag="sums", name="sums")
                nc.gpsimd.memset(sums[:], 1.0)
                for h2 in range(2):
                    for tw in range(2):
                        r = 32 * (2 * tw + h2)
                        nc.vector.tensor_copy(
                            sums[r : r + 1, :], get(h2, tw)[DH : DH + 1, :]
                        )
                recips = sums_pool.tile([97, 512], f32, tag="recips", name="recips")
                nc.vector.reciprocal_approx_fast(recips[:], sums[:])
                rbs = {}
                for tw in range(2):
                    for h2 in range(2):
                        r = 32 * (2 * tw + h2)
                        r_t = r_pool.tile([1, 512], f32, tag="r", name="r_t")
                        nc.vector.tensor_copy(r_t[0:1, :], recips[r : r + 1, :])
                        rb_t = rb_pool.tile([64, 512], f32, tag="rb", name="rb_t")
                        nc.gpsimd.partition_broadcast(rb_t[:], r_t[0:1, :])
                        rbs[(h2, tw)] = rb_t
                return rbs

            def emit_norm_muls(th_, pp_, get, rbs):
                for tw in range(2):
                    for h2 in range(2):
                        tcol = th_ * 1024 + tw * 512
                        nc.vector.tensor_mul(
                            outT_sb[h2 * 64 : (h2 + 1) * 64, pp_, tcol : tcol + 512],
                            get(h2, tw)[0:DH, :],
                            rbs[(h2, tw)],
                        )

            def emit_wo(th_, tts=None):
                for tt in tts if tts is not None else range(th_ * 8, (th_ + 1) * 8):
                    ps = psA.tile([128, 512], f32, tag="mm", name="y_ps")
                    for c in range(HE // 128):
                        nc.tensor.matmul(
                            ps[:],
                            lhsT=outT_sb[:, c, tt * 128 : (tt + 1) * 128],
                            rhs=wo_sb[:, c, :],
                            start=(c == 0),
                            stop=(c == HE // 128 - 1),
                        )
                    y_sb = y_pool.tile([128, 512], f32, tag="y", name="y_sb")
                    nc.vector.tensor_copy(y_sb[:], ps[:])
                    nc.sync.dma_start(y[tt * 128 : (tt + 1) * 128, :], y_sb[:])

            # ---- prologue: projections chase the DMA slices --------------
            emit_kt(0, [0])
            emit_qt(0, 0, (0,))
            emit_kt(0, [1])
            emit_qt(0, 0, (1,))
            emit_kt(0, list(range(2, len(kv_slices))))

            # ---- phases: pp0 first so kt1/qt1 projections spread out -----
            phases = [(0, 0), (1, 0), (0, 1), (1, 1)]
            pending = None

            filler_plan = {}

            def add_filler(ph, st, fn):
                filler_plan.setdefault((ph, min(st, nkt - 1)), []).append(fn)

            v_jobs = [(lambda vst=vst: emit_v(vst)) for vst in range(nkt)]
            slots0 = max(min(5, nkt - 1), 1)
            per = (len(v_jobs) + slots0 - 1) // slots0
            for si in range(slots0):
                for fn in v_jobs[si * per : (si + 1) * per]:
                    add_filler(0, si, fn)
            add_filler(0, 5, lambda: emit_qt(0, 1))
            add_filler(1, 2, lambda: emit_kt(1))
            add_filler(1, 3, lambda: emit_qt(1, 0))
            add_filler(2, 2, lambda: emit_qt(1, 1))
            add_filler(3, 6, lambda: emit_wo(0, range(0, 4)))
            add_filler(3, 7, lambda: emit_wo(0, range(4, 8)))
            av_st = {
                0: min(5, nkt - 1),
                1: min(3, nkt - 1),
                2: min(2, nkt - 1),
                3: min(7, nkt - 1),
            }

            for phase_i, (th, pp) in enumerate(phases):
                av_l = None
                backlog = []

                def ensure_av():
                    nonlocal av_l
                    if av_l is not None:
                        return
                    av_l = [
                        [
                            psA.tile(
                                [128, 512], f32, tag="mm", name=f"av{h2}_{tw}"
                            )
                            for tw in range(2)
                        ]
                        for h2 in range(2)
                    ]
                    for at_, st_, tw_ in backlog:
                        emit_attnv(av_l, at_, st_, pp, tw_)
                    backlog.clear()

                staged = None
                rbs = None
                for st in range(nkt):
                    ats = emit_scores(pp, th, st)
                    for tw, at in enumerate(ats):
                        if av_l is None:
                            backlog.append((at, st, tw))
                        else:
                            emit_attnv(av_l, at, st, pp, tw)
                    if st == 0 and pending is not None:
                        staged = emit_stage(pending[2])
                    if st == 1 and pending is not None:
                        rbs = emit_norm_recip(lambda h2, tw: staged[(h2, tw)])
                    if st == min(4, nkt - 1) and pending is not None:
                        emit_norm_muls(
                            pending[0], pending[1],
                            lambda h2, tw: staged[(h2, tw)], rbs,
                        )
                        pending = None
                    for fn in filler_plan.get((phase_i, st), []):
                        fn()
                    if st == av_st[phase_i]:
                        ensure_av()
                ensure_av()
                pending = (th, pp, av_l)

            # ---- tail: last phase normalize + Wo -------------------------
            # PE is idle here: broadcast recip rows via a ones[1,64] matmul
            # into free sc-tag psum instead of 4 serial gpsimd broadcasts.
            # Batched emission (copies, then broadcasts, then muls) so the
            # DVE/PE queues never ping-pong.
            th_, pp_, av_ = pending
            sums = sums_pool.tile([97, 512], f32, tag="sums", name="sums")
            nc.gpsimd.memset(sums[:], 1.0)
            for h2 in range(2):
                for tw in range(2):
                    r = 32 * (2 * tw + h2)
                    nc.vector.tensor_copy(
                        sums[r : r + 1, :], av_[h2][tw][DH : DH + 1, :]
                    )
            recips = sums_pool.tile([97, 512], f32, tag="recips", name="recips")
            nc.vector.reciprocal_approx_fast(recips[:], sums[:])
            r_bfs = {}
            for tw in range(2):
                for h2 in range(2):
                    r = 32 * (2 * tw + h2)
                    r_bf = r_pool.tile([1, 512], bf16, tag="rbf", name="r_bf")
                    nc.vector.tensor_copy(r_bf[0:1, :], recips[r : r + 1, :])
                    r_bfs[(h2, tw)] = r_bf
            tail_staged = emit_stage(av_)
            rb_pss = {}
            for tw in range(2):
                for h2 in range(2):
                    rb_ps = psS.tile([64, 512], f32, tag="sc", name="rb_ps")
                    nc.tensor.matmul(
                        rb_ps[:], lhsT=ones_sb[:], rhs=r_bfs[(h2, tw)][0:1, :],
                        start=True, stop=True,
                    )
                    rb_pss[(h2, tw)] = rb_ps
            for tw in range(2):
                for h2 in range(2):
                    tcol = th_ * 1024 + tw * 512
                    nc.vector.tensor_mul(
                        outT_sb[h2 * 64 : (h2 + 1) * 64, pp_, tcol : tcol + 512],
                        tail_staged[(h2, tw)][0:DH, :],
                        rb_pss[(h2, tw)][:],
                    )
                emit_wo(th_, tts=range(th_ * 8 + tw * 4, th_ * 8 + (tw + 1) * 4))

    nc.compile()
    return nc


_NC_CACHE = {}
_LAST_STATE = {}


def _get_nc(nkt=None):
    if nkt is None:
        nkt = _LAST_STATE.get("nkt", 9)
    if nkt not in _NC_CACHE:
        _NC_CACHE[nkt] = build_nc(nkt)
    return _NC_CACHE[nkt]


def make_in_maps(x, mask, Wq, Wk, Wv, Wo):
    bf = ml_dtypes.bfloat16
    mask = np.asarray(mask)
    perms = []
    counts = []
    for b in range(B):
        valid = np.flatnonzero(mask[b] > 0)
        invalid = np.flatnonzero(mask[b] <= 0)
        perms.append(np.concatenate([valid, invalid]).astype(np.int64))
        counts.append(len(valid))
    nkt = max(1, int(np.ceil(max(counts) / 128)))
    nkt = min(nkt, S // 128)
    SV = nkt * 128
    _LAST_STATE["nkt"] = nkt
    _LAST_STATE["perms"] = perms

    # [H, D, DH] -> [D, H*DH]
    wq_f = np.ascontiguousarray(Wq.transpose(1, 0, 2).reshape(D, H * DH))
    wk_f = np.ascontiguousarray(Wk.transpose(1, 0, 2).reshape(D, H * DH))
    wv_f = np.ascontiguousarray(Wv.transpose(1, 0, 2).reshape(D, H * DH))
    in_maps = []
    for c in range(N_CORES):
        b, hg = c // 2, c % 2
        perm = perms[b]
        xT_p = np.ascontiguousarray(x[b].T[:, perm]).astype(bf)  # [D, S] permuted
        mb = np.where(mask[b][perm] > 0, 0.0, -MASK_NUM).astype(np.float32)[:SV]
        cols = slice(hg * HE, (hg + 1) * HE)
        in_maps.append(
            {
                "xT": xT_p,
                "wq": np.ascontiguousarray(wq_f[:, cols]).astype(bf),
                "wk": np.ascontiguousarray(wk_f[:, cols]).astype(bf),
                "wv": np.ascontiguousarray(wv_f[:, cols]).astype(bf),
                "wo": np.ascontiguousarray(Wo[cols, :]).astype(bf),
                "mbias": np.ascontiguousarray(mb),
            }
        )
    return in_maps


def combine_results(results):
    perms = _LAST_STATE["perms"]
    y = np.zeros((B, S, D), np.float32)
    for b in range(B):
        yp = results[2 * b]["y"] + results[2 * b + 1]["y"]
        y[b][perms[b]] = yp
    return y


def kernel(x, mask, Wq, Wk, Wv, Wo):
    in_maps = make_in_maps(
        np.asarray(x, np.float32),
        np.asarray(mask),
        np.asarray(Wq, np.float32),
        np.asarray(Wk, np.float32),
        np.asarray(Wv, np.float32),
        np.asarray(Wo, np.float32),
    )
    nc = _get_nc(_LAST_STATE["nkt"])
    res = run_bass_kernel_spmd(nc, in_maps, core_ids=list(range(N_CORES)))
    return combine_results(res.results)


# revision 22
# speedup vs baseline: 1.4274x; 1.0273x over previous
"""Trainium2 Bass kernel for nn_Encoder (B=4, S=2048, D=512, H=8 self-attention).

Sharding over 8 NeuronCores: core c -> (batch b = c//2, head-group hg = c%2).
Each core computes, for its batch and its 4 heads, the full attention block
plus a partial output projection y_part = attn_out @ Wo[group rows]. The host
sums the two partial y tensors per batch.

Key compaction: the key-padding mask kills ~half the keys (their attention
weight is exactly exp(-1e9) = 0). The host permutes each batch's sequence so
valid keys come first; K/V projections, scores, exp and attnV run only over
NKT = ceil(n_valid/128) key tiles instead of 16. Queries stay full (the
output rows are un-permuted on the host).

Device-side layout (contraction dim always on SBUF partitions):
  xT_kv/xT_q*       : host-permuted input, d on partitions; separate tiles per
                      consumer so a later DMA never carries a WAR hazard
                      against projection reads
  KT/QT per pair    : [128, SV] / [128, 1024/th] = [2 heads' e, s/t]
  scores sc[tw]     : [s, {h0 512t | h1 512t}] -> the two heads' matmuls for a
                      given t-window write ONE psum tile, so the scheduler
                      keeps the (row_grp=0, row_grp=64) pair adjacent
  V' [s, e + ones]  : appended ones column makes the softmax denominator fall
                      out of the attnV matmul (psum row 64) for free
  outT [he, t]      : exactly the lhsT layout the Wo projection wants

ACT (exp) is the bottleneck engine; phases are paced so it never starves:
projection/Wo psums share the av tag and are scheduled into per-phase slack,
attnV drains through an `at` backlog, normalize reads av psum directly and
uses reciprocal_approx_fast, and the tail broadcasts recip via a PE matmul
(PE is idle there) instead of 4 serial gpsimd broadcasts.
"""

import ml_dtypes
import numpy as np

import concourse.mybir as mybir
import concourse.tile as tile
from concourse import bacc
from concourse.bass_utils import run_bass_kernel_spmd

B, S, D, H = 4, 2048, 512, 8
DH = D // H          # 64
HPC = H // 2         # 4 heads per core
HE = HPC * DH        # 256 output-proj rows per core
T = S                # full query length per core
NDC = D // 128       # 4 contraction chunks for projections
MASK_NUM = 1.0e9
N_CORES = 8

f32 = mybir.dt.float32
bf16 = mybir.dt.bfloat16
EXP = mybir.ActivationFunctionType.Exp
SCALE = float(1.0 / np.sqrt(DH))


def build_nc(nkt):
    SV = nkt * 128
    nc = bacc.Bacc("TRN2", target_bir_lowering=False, debug=False, num_devices=1)

    xT = nc.dram_tensor("xT", [D, S], bf16, kind="ExternalInput").ap()
    wq = nc.dram_tensor("wq", [D, HE], bf16, kind="ExternalInput").ap()
    wk = nc.dram_tensor("wk", [D, HE], bf16, kind="ExternalInput").ap()
    wv = nc.dram_tensor("wv", [D, HE], bf16, kind="ExternalInput").ap()
    wo = nc.dram_tensor("wo", [HE, D], bf16, kind="ExternalInput").ap()
    mb = nc.dram_tensor("mbias", [SV], f32, kind="ExternalInput").ap()
    y = nc.dram_tensor("y", [T, D], f32, kind="ExternalOutput").ap()

    with tile.TileContext(nc) as tc:
        with (
            tc.tile_pool(name="const", bufs=1) as const,
            tc.tile_pool(name="psS", bufs=2, space="PSUM") as psS,
            tc.tile_pool(name="psA", bufs=4, space="PSUM") as psA,
            tc.tile_pool(name="attnT", bufs=18) as at_pool,
            tc.tile_pool(name="yout", bufs=4) as y_pool,
            tc.tile_pool(name="recip", bufs=4) as r_pool,
            tc.tile_pool(name="recipb", bufs=4) as rb_pool,
            tc.tile_pool(name="sums", bufs=2) as sums_pool,
            tc.tile_pool(name="avsb", bufs=8) as avsb_pool,
        ):
            # ---- HAM warmup: keep PE busy during the DMA prologue so the
            # clock gate is at 8/8 when real matmuls arrive.
            warm_sb = const.tile([128, 512], bf16, tag="warm")
            nc.gpsimd.memset(warm_sb[:], 0.0)
            warm_ps = psA.tile([128, 512], f32, tag="mm", name="warm_ps")
            for _ in range(18):
                nc.tensor.matmul(
                    warm_ps[:], lhsT=warm_sb[:, 0:128], rhs=warm_sb[:],
                    start=True, stop=True,
                )

            # ---- ACT table preload: a dummy exp so the Exp spline tables DMA
            # in during the prologue instead of before the first real exp.
            warm_act = const.tile([1, 2], f32, tag="warm_act")
            nc.gpsimd.memset(warm_act[:], 0.0)
            nc.scalar.activation(warm_act[0:1, 0:1], warm_act[0:1, 1:2], EXP)

            # ---- DMA loads, critical-path order -------------------------
            # Column-sliced into separate tiles so the first projections can
            # start as soon as their slice lands (dep granularity is per-tile).
            kv_slices = []
            c0 = 0
            while c0 < SV:
                w = min(512, SV - c0)
                kv_slices.append((c0, w))
                c0 += w
            xT_kv = [
                const.tile([128, NDC, w], bf16, tag=f"xTkv{j}", name=f"xTkv{j}")
                for j, (c0, w) in enumerate(kv_slices)
            ]
            xT_q = [
                [
                    const.tile(
                        [128, NDC, 512], bf16, tag=f"xTq{th}{i}", name=f"xTq{th}{i}"
                    )
                    for i in range(2)
                ]
                for th in range(2)
            ]
            wq_sb = const.tile([128, NDC, HE], bf16, tag="wq")
            wk_sb = const.tile([128, NDC, HE], bf16, tag="wk")
            wv_sb = const.tile([128, NDC, HE], bf16, tag="wv")
            wo_sb = const.tile([128, HE // 128, D], bf16, tag="wo")
            mb_sb = const.tile([128, nkt], f32, tag="mb")
            xT_r = xT.rearrange("(c p) s -> c p s", p=128)

            def dma_kv(j):
                c0, w = kv_slices[j]
                for dc in range(NDC):
                    nc.sync.dma_start(xT_kv[j][:, dc, :], xT_r[dc][:, c0 : c0 + w])

            def dma_q(th, i):
                t0 = th * 1024 + i * 512
                for dc in range(NDC):
                    nc.sync.dma_start(
                        xT_q[th][i][:, dc, :], xT_r[dc][:, t0 : t0 + 512]
                    )

            nc.sync.dma_start(wk_sb[:], wk.rearrange("(c p) n -> p c n", p=128))
            dma_kv(0)
            nc.sync.dma_start(wq_sb[:], wq.rearrange("(c p) n -> p c n", p=128))
            dma_q(0, 0)
            nc.sync.dma_start(mb_sb[:], mb.rearrange("(j p) -> p j", p=128))
            dma_kv(1)
            dma_q(0, 1)
            nc.sync.dma_start(wv_sb[:], wv.rearrange("(c p) n -> p c n", p=128))
            for j in range(2, len(kv_slices)):
                dma_kv(j)
            dma_q(1, 0)
            dma_q(1, 1)
            nc.sync.dma_start(wo_sb[:], wo.rearrange("(c p) n -> p c n", p=128))

            # V' tiles: [s-tile][local head][DH + ones column]
            v_sb = const.tile([128, nkt, HPC, DH + 1], bf16, tag="v")
            nc.gpsimd.memset(v_sb[:, :, :, DH : DH + 1], 1.0)
            ones_sb = const.tile([1, DH], bf16, tag="ones")
            nc.gpsimd.memset(ones_sb[:], 1.0)

            kt_sb = [
                const.tile([128, SV], bf16, tag=f"kt{pp}", name=f"kt{pp}")
                for pp in range(2)
            ]
            qt_sb = [
                const.tile([128, T], bf16, tag=f"qt{pp}", name=f"qt{pp}")
                for pp in range(2)
            ]
            outT_sb = const.tile([128, HE // 128, T], bf16, tag="outT")

            # ---- projection emitters (psum shares the "mm" tag) ----------
            def emit_kt(pp, slices=None):
                for j in slices if slices is not None else range(len(kv_slices)):
                    c0, w = kv_slices[j]
                    ps = psA.tile([128, 512], f32, tag="mm", name="kproj_ps")
                    for dc in range(NDC):
                        nc.tensor.matmul(
                            ps[:, 0:w],
                            lhsT=wk_sb[:, dc, pp * 128 : (pp + 1) * 128],
                            rhs=xT_kv[j][:, dc, :],
                            start=(dc == 0),
                            stop=(dc == NDC - 1),
                        )
                    nc.vector.tensor_copy(kt_sb[pp][:, c0 : c0 + w], ps[:, 0:w])

            def emit_qt(pp, th, sc_is=(0, 1)):
                for sc_i in sc_is:
                    ps = psA.tile([128, 512], f32, tag="mm", name="qproj_ps")
                    for dc in range(NDC):
                        nc.tensor.matmul(
                            ps[:],
                            lhsT=wq_sb[:, dc, pp * 128 : (pp + 1) * 128],
                            rhs=xT_q[th][sc_i][:, dc, :],
                            start=(dc == 0),
                            stop=(dc == NDC - 1),
                        )
                    t0 = th * 1024 + sc_i * 512
                    nc.vector.tensor_copy(qt_sb[pp][:, t0 : t0 + 512], ps[:])

            def emit_v(vst):
                j, r = vst // 4, vst % 4
                ps = psA.tile([128, 512], f32, tag="mm", name="vproj_ps")
                for dc in range(NDC):
                    nc.tensor.matmul(
                        ps[:, 0:HE],
                        lhsT=xT_kv[j][:, dc, r * 128 : (r + 1) * 128],
                        rhs=wv_sb[:, dc, :],
                        start=(dc == 0),
                        stop=(dc == NDC - 1),
                    )
                nc.vector.tensor_copy(
                    v_sb[:, vst, :, 0:DH],
                    ps[:, 0:HE].rearrange("p (h e) -> p h e", e=DH),
                )

            # ---- attention emitters --------------------------------------
            # sc tile per (st, tw): cols 0:512 = h0, 512:1024 = h1. The two
            # heads' matmuls write one tile -> adjacent in the PE queue ->
            # concurrent in row groups 0/64.
            def emit_scores(pp, th, st):
                ats = []
                for tw in range(2):
                    scp = psS.tile([128, 1024], f32, tag="sc", name=f"sc{tw}")
                    tcol = th * 1024 + tw * 512
                    for h2 in range(2):
                        off = h2 * 64
                        nc.tensor.matmul(
                            scp[:, h2 * 512 : (h2 + 1) * 512],
                            lhsT=kt_sb[pp][off : off + 64, st * 128 : (st + 1) * 128],
                            rhs=qt_sb[pp][off : off + 64, tcol : tcol + 512],
                            start=True,
                            stop=True,
                        )
                    at = at_pool.tile([128, 1024], bf16, tag="at", name="at")
                    nc.scalar.activation(
                        at[:], scp[:], EXP,
                        bias=mb_sb[:, st : st + 1], scale=SCALE,
                    )
                    ats.append(at)
                return ats

            def emit_attnv(av_l, at, st, pp, tw):
                for h2 in range(2):
                    h = 2 * pp + h2
                    nc.tensor.matmul(
                        av_l[h2][tw][0 : DH + 1, :],
                        lhsT=v_sb[:, st, h, :],
                        rhs=at[:, h2 * 512 : (h2 + 1) * 512],
                        start=(st == 0),
                        stop=(st == nkt - 1),
                    )

            # stage the pending phase's av psum to SBUF: releases the psA
            # slots immediately so boundary fillers never head-block the PE
            # queue behind the (slow, gpsimd-serial) normalize chain
            def emit_stage(av_):
                staged = {}
                for h2 in range(2):
                    for tw in range(2):
                        s = avsb_pool.tile(
                            [DH + 1, 512], f32, tag="avsb", name="av_sb"
                        )
                        nc.vector.tensor_copy(s[:], av_[h2][tw][0 : DH + 1, :])
                        staged[(h2, tw)] = s
                return staged

            # Boundary normalize, split in two emission points: the recip +
            # gpsimd broadcasts go early (st1); the DVE multiplies are
            # emitted at st4 so the boundary projections' psum-drain copies
            # queue AHEAD of them in the DVE FIFO — by st4 the broadcasts are
            # long done and the muls never head-block anything.
            def emit_norm_recip(get):
                sums = sums_pool.tile([97, 512], f32, tag="sums", name="sums")
                nc.gpsimd.memset(sums[:], 1.0)
                for h2 in range(2):
                    for tw in range(2):
                        r = 32 * (2 * tw + h2)
                        nc.vector.tensor_copy(
                            sums[r : r + 1, :], get(h2, tw)[DH : DH + 1, :]
                        )
                recips = sums_pool.tile([97, 512], f32, tag="recips", name="recips")
                nc.vector.reciprocal_approx_fast(recips[:], sums[:])
                rbs = {}
                for tw in range(2):
                    for h2 in range(2):
                        r = 32 * (2 * tw + h2)
                        r_t = r_pool.tile([1, 512], f32, tag="r", name="r_t")
                        nc.vector.tensor_copy(r_t[0:1, :], recips[r : r + 1, :])
                        rb_t = rb_pool.tile([64, 512], f32, tag="rb", name="rb_t")
                        nc.gpsimd.partition_broadcast(rb_t[:], r_t[0:1, :])
                        rbs[(h2, tw)] = rb_t
                return rbs

            def emit_norm_muls(th_, pp_, get, rbs):
                for tw in range(2):
                    for h2 in range(2):
                        tcol = th_ * 1024 + tw * 512
                        nc.vector.tensor_mul(
                            outT_sb[h2 * 64 : (h2 + 1) * 64, pp_, tcol : tcol + 512],
                            get(h2, tw)[0:DH, :],
                            rbs[(h2, tw)],
                        )

            def emit_wo(th_, tts=None):
                for tt in tts if tts is not None else range(th_ * 8, (th_ + 1) * 8):
                    ps = psA.tile([128, 512], f32, tag="mm", name="y_ps")
                    for c in range(HE // 128):
                        nc.tensor.matmul(
                            ps[:],
                            lhsT=outT_sb[:, c, tt * 128 : (tt + 1) * 128],
                            rhs=wo_sb[:, c, :],
                            start=(c == 0),
                            stop=(c == HE // 128 - 1),
                        )
                    y_sb = y_pool.tile([128, 512], f32, tag="y", name="y_sb")
                    nc.vector.tensor_copy(y_sb[:], ps[:])
                    nc.sync.dma_start(y[tt * 128 : (tt + 1) * 128, :], y_sb[:])

            # ---- prologue: projections chase the DMA slices --------------
            emit_kt(0, [0])
            emit_qt(0, 0, (0,))
            emit_kt(0, [1])
            emit_qt(0, 0, (1,))
            emit_kt(0, list(range(2, len(kv_slices))))

            # ---- phases: pp0 first so kt1/qt1 projections spread out -----
            phases = [(0, 0), (1, 0), (0, 1), (1, 1)]
            pending = None

            filler_plan = {}

            def add_filler(ph, st, fn):
                filler_plan.setdefault((ph, min(st, nkt - 1)), []).append(fn)

            v_jobs = [(lambda vst=vst: emit_v(vst)) for vst in range(nkt)]
            slots0 = max(min(5, nkt - 1), 1)
            per = (len(v_jobs) + slots0 - 1) // slots0
            for si in range(slots0):
                for fn in v_jobs[si * per : (si + 1) * per]:
                    add_filler(0, si, fn)
            add_filler(0, 5, lambda: emit_qt(0, 1))
            add_filler(1, 2, lambda: emit_kt(1))
            add_filler(1, 3, lambda: emit_qt(1, 0))
            add_filler(2, 2, lambda: emit_qt(1, 1))
            add_filler(3, 6, lambda: emit_wo(0, range(0, 4)))
            add_filler(3, 7, lambda: emit_wo(0, range(4, 8)))
            av_st = {
                0: min(5, nkt - 1),
                1: min(3, nkt - 1),
                2: min(2, nkt - 1),
                3: min(7, nkt - 1),
            }

            for phase_i, (th, pp) in enumerate(phases):
                av_l = None
                backlog = []

                def ensure_av():
                    nonlocal av_l
                    if av_l is not None:
                        return
                    av_l = [
                        [
                            psA.tile(
                                [128, 512], f32, tag="mm", name=f"av{h2}_{tw}"
                            )
                            for tw in range(2)
                        ]
                        for h2 in range(2)
                    ]

                def drain(k):
                    for at_, st_, tw_ in backlog[:k]:
                        emit_attnv(av_l, at_, st_, pp, tw_)
                    del backlog[:k]

                staged = None
                rbs = None
                for st in range(nkt):
                    ats = emit_scores(pp, th, st)
                    for tw, at in enumerate(ats):
                        backlog.append((at, st, tw))
                    if av_l is not None:
                        drain(6)
                    if st == 0 and pending is not None:
                        staged = emit_stage(pending[2])
                    if st == 1 and pending is not None:
                        rbs = emit_norm_recip(lambda h2, tw: staged[(h2, tw)])
                    if st == min(4, nkt - 1) and pending is not None:
                        emit_norm_muls(
                            pending[0], pending[1],
                            lambda h2, tw: staged[(h2, tw)], rbs,
                        )
                        pending = None
                    for fn in filler_plan.get((phase_i, st), []):
                        fn()
                    if st == av_st[phase_i]:
                        ensure_av()
                        drain(6)
                ensure_av()
                drain(len(backlog))
                pending = (th, pp, av_l)

            # ---- tail: last phase normalize + Wo -------------------------
            # PE is idle here: broadcast recip rows via a ones[1,64] matmul
            # into free sc-tag psum instead of 4 serial gpsimd broadcasts.
            # Batched emission (copies, then broadcasts, then muls) so the
            # DVE/PE queues never ping-pong.
            th_, pp_, av_ = pending
            sums = sums_pool.tile([97, 512], f32, tag="sums", name="sums")
            nc.gpsimd.memset(sums[:], 1.0)
            for h2 in range(2):
                for tw in range(2):
                    r = 32 * (2 * tw + h2)
                    nc.vector.tensor_copy(
                        sums[r : r + 1, :], av_[h2][tw][DH : DH + 1, :]
                    )
            recips = sums_pool.tile([97, 512], f32, tag="recips", name="recips")
            nc.vector.reciprocal_approx_fast(recips[:], sums[:])
            r_bfs = {}
            for tw in range(2):
                for h2 in range(2):
                    r = 32 * (2 * tw + h2)
                    r_bf = r_pool.tile([1, 512], bf16, tag="rbf", name="r_bf")
                    nc.vector.tensor_copy(r_bf[0:1, :], recips[r : r + 1, :])
                    r_bfs[(h2, tw)] = r_bf
            tail_staged = emit_stage(av_)
            rb_pss = {}
            for tw in range(2):
                for h2 in range(2):
                    rb_ps = psS.tile([64, 512], f32, tag="sc", name="rb_ps")
                    nc.tensor.matmul(
                        rb_ps[:], lhsT=ones_sb[:], rhs=r_bfs[(h2, tw)][0:1, :],
                        start=True, stop=True,
                    )
                    rb_pss[(h2, tw)] = rb_ps
            for tw in range(2):
                for h2 in range(2):
                    tcol = th_ * 1024 + tw * 512
                    nc.vector.tensor_mul(
                        outT_sb[h2 * 64 : (h2 + 1) * 64, pp_, tcol : tcol + 512],
                        tail_staged[(h2, tw)][0:DH, :],
                        rb_pss[(h2, tw)][:],
                    )
            emit_wo(th_)

    nc.compile()
    return nc


_NC_CACHE = {}
_LAST_STATE = {}


def _get_nc(nkt=None):
    if nkt is None:
        nkt = _LAST_STATE.get("nkt", 9)
    if nkt not in _NC_CACHE:
        _NC_CACHE[nkt] = build_nc(nkt)
    return _NC_CACHE[nkt]


def make_in_maps(x, mask, Wq, Wk, Wv, Wo):
    bf = ml_dtypes.bfloat16
    mask = np.asarray(mask)
    perms = []
    counts = []
    for b in range(B):
        valid = np.flatnonzero(mask[b] > 0)
        invalid = np.flatnonzero(mask[b] <= 0)
        perms.append(np.concatenate([valid, invalid]).astype(np.int64))
        counts.append(len(valid))
    nkt = max(1, int(np.ceil(max(counts) / 128)))
    nkt = min(nkt, S // 128)
    SV = nkt * 128
    _LAST_STATE["nkt"] = nkt
    _LAST_STATE["perms"] = perms

    # [H, D, DH] -> [D, H*DH]
    wq_f = np.ascontiguousarray(Wq.transpose(1, 0, 2).reshape(D, H * DH))
    wk_f = np.ascontiguousarray(Wk.transpose(1, 0, 2).reshape(D, H * DH))
    wv_f = np.ascontiguousarray(Wv.transpose(1, 0, 2).reshape(D, H * DH))
    in_maps = []
    for c in range(N_CORES):
        b, hg = c // 2, c % 2
        perm = perms[b]
        xT_p = np.ascontiguousarray(x[b].T[:, perm]).astype(bf)  # [D, S] permuted
        mb = np.where(mask[b][perm] > 0, 0.0, -MASK_NUM).astype(np.float32)[:SV]
        cols = slice(hg * HE, (hg + 1) * HE)
        in_maps.append(
            {
                "xT": xT_p,
                "wq": np.ascontiguousarray(wq_f[:, cols]).astype(bf),
                "wk": np.ascontiguousarray(wk_f[:, cols]).astype(bf),
                "wv": np.ascontiguousarray(wv_f[:, cols]).astype(bf),
                "wo": np.ascontiguousarray(Wo[cols, :]).astype(bf),
                "mbias": np.ascontiguousarray(mb),
            }
        )
    return in_maps


def combine_results(results):
    perms = _LAST_STATE["perms"]
    y = np.zeros((B, S, D), np.float32)
    for b in range(B):
        yp = results[2 * b]["y"] + results[2 * b + 1]["y"]
        y[b][perms[b]] = yp
    return y


def kernel(x, mask, Wq, Wk, Wv, Wo):
    in_maps = make_in_maps(
        np.asarray(x, np.float32),
        np.asarray(mask),
        np.asarray(Wq, np.float32),
        np.asarray(Wk, np.float32),
        np.asarray(Wv, np.float32),
        np.asarray(Wo, np.float32),
    )
    nc = _get_nc(_LAST_STATE["nkt"])
    res = run_bass_kernel_spmd(nc, in_maps, core_ids=list(range(N_CORES)))
    return combine_results(res.results)
